# revision 49
# baseline (speedup 1.0000x reference)
"""DyConv (MoE-routed dynamic convolution) Trainium2 Bass kernel.

Data-parallel over batch: 32 samples -> 8 cores x 4 samples.
Per sample, fully on-device:
  gap  = mean(x, HW)                  (VectorE reduce, 1/HW folded into w1)
  h    = relu(gap @ w1.T + b1)        (TensorE matmul K=Cin=128, ScalarE relu)
  l    = h @ w2.T                     (TensorE matmul K=16)
  r    = softmax(l/30 + b2/30)        (ScalarE exp, b2 as const factor
                                       exp(b2/30) on DVE, DVE recip+scale)
  kern = sum_e r[e] * convs[e]        (VectorE scalar_tensor_tensor FMAs)
  out  = conv2d(x, kern, pad=1)       (9 shifted float32r matmuls accumulated
                                       in PSUM; Cin=128 partition contraction,
                                       Cout = 2 halves of 128, 7 row tiles of
                                       N=8*56=448)

The emission is software-pipelined: sample b+1's producer work (image load,
GAP, router, expert mixing) is emitted between sample b's two conv halves so
DVE/ACT prep overlaps PE matmuls. float32r (fp32 rounded to 11 mantissa
bits, TF32-like) runs the PE at 1 cycle/row — 4x faster than fp32 — with
~2e-4 relative output error.
"""

import os
from contextlib import ExitStack

import numpy as np

import concourse.bass as bass
import concourse.bacc as bacc
import concourse.tile as tile
from concourse import mybir
from concourse.bass_utils import run_bass_kernel_spmd

F32 = mybir.dt.float32
F32R = mybir.dt.float32r

B, CIN, H, W = 32, 128, 56, 56
COUT, KS, E, R = 256, 3, 4, 16
NCORES = 8
BL = B // NCORES  # samples per core
TEMP = 30.0
HP, WP = H + 2, W + 2  # zero-padded image dims in SBUF
HWN = H * W  # 3136
ROWS_PER_TILE = 8
NTILES = H // ROWS_PER_TILE  # 7
NFREE = ROWS_PER_TILE * W  # 448 fp32 <= 512 (one PSUM bank)
TAPCO = KS * KS * COUT  # 2304, per-expert slice [tap, co]

# taps in kh-major order, matching the [ci, e, kh, kw, co] host layout
TAPS = [(dh, dw) for dh in (-1, 0, 1) for dw in (-1, 0, 1)]

# module-level knobs for test.py
TRACE = os.environ.get("DYCONV_TRACE", "0") == "1"
LAST_RESULTS = None
MM_DTYPE = F32R
# benchmarking: wrap the whole kernel body in a For_i loop of this many
# iterations (one NEFF, repeated device-side) so wall-clock timing is
# dominated by device time, not axon dispatch RTT.
LOOP_REPS = int(os.environ.get("DYCONV_LOOP_REPS", "1"))
# conv rhs addressing: 0 = strided [8 rows x 56] views of the padded image,
# 1 = fully contiguous 464-element windows over flat padded rows (guard
# elements at both ends keep all 9 tap offsets in-bounds)
CONTIG_RHS = os.environ.get("DYCONV_CONTIG", "0") == "1"
NFREE_C = ROWS_PER_TILE * WP  # 464 fp32 <= 512


def _build_program():
    # Bacc (not raw Bass): its compile() runs move_matmul_waits_to_ldweights
    # + generate_event_semaphores, legalizing instructions that need more
    # than one hardware sync-wait slot.
    nc = bacc.Bacc("TRN2", target_bir_lowering=False, debug=False)
    # x and convs feed float32r matmuls; host pre-rounds both to the fp32r
    # grid (RNE to 11 mantissa bits) so every on-chip conversion to f32r is
    # value-preserving. The on-chip f32r producers (DVE copy / mixing) are
    # what satisfies the BIR verifier's rounded-producer rule.
    x_d = nc.dram_tensor("x", [BL, CIN, H, W], F32, kind="ExternalInput").ap()
    # host-prearranged: convs_r[ci, e, kh, kw, co] flattened to [128, E*9*COUT]
    convs_d = nc.dram_tensor("convs", [CIN, E * TAPCO], F32R, kind="ExternalInput").ap()
    # w1.T / (H*W)  -> [CIN, R]
    w1t_d = nc.dram_tensor("w1t", [CIN, R], F32, kind="ExternalInput").ap()
    b1_d = nc.dram_tensor("b1", [R, 1], F32, kind="ExternalInput").ap()
    # w2.T -> [R, E]; g = exp(b2/TEMP) -> [1, E]: the bias enters softmax
    # as a constant per-expert multiplicative factor applied after exp.
    w2t_d = nc.dram_tensor("w2t", [R, E], F32, kind="ExternalInput").ap()
    g_d = nc.dram_tensor("g", [1, E], F32, kind="ExternalInput").ap()
    out_d = nc.dram_tensor("out", [BL, COUT, H, W], F32, kind="ExternalOutput").ap()

    with tile.TileContext(nc) as tc, ExitStack() as ctx:
        if LOOP_REPS > 1:
            with tc.For_i(0, LOOP_REPS, 1, hint_engines=(mybir.EngineType.PE,)):
                _emit(ctx, tc, x_d, convs_d, w1t_d, b1_d, w2t_d, g_d, out_d)
        else:
            _emit(ctx, tc, x_d, convs_d, w1t_d, b1_d, w2t_d, g_d, out_d)
    nc.compile()
    return nc


def _emit(ctx, tc, x_d, convs_d, w1t_d, b1_d, w2t_d, g_d, out_d):
    # The fp32/fp32r matmul hardware encoding (fused 4-byte weight load,
    # S3_LW) carries at most ONE sync wait, so every matmul is arranged to
    # depend on a single engine's semaphore:
    #   conv matmuls  -> DVE only (kern mixing, padded-image copy, PSUM
    #                    bank release via DVE stage copies)
    #   router mm1/mm2-> ACT only (gap via ACT accumulate, relu on ACT)
    #   rb broadcast  -> DVE only
    # One-time DMA waits for the router weights are absorbed by warmup
    # matmuls that read only those tiles.
    nc = tc.nc

    const_pool = ctx.enter_context(tc.tile_pool(name="const", bufs=1))
    xp_pool = ctx.enter_context(tc.tile_pool(name="xpad", bufs=3))
    kern_pool = ctx.enter_context(tc.tile_pool(name="kern", bufs=2))
    small_pool = ctx.enter_context(tc.tile_pool(name="small", bufs=2))
    stage_pool = ctx.enter_context(tc.tile_pool(name="stage", bufs=4))
    psum_pool = ctx.enter_context(tc.tile_pool(name="psum", bufs=3, space="PSUM"))
    psum_r_pool = ctx.enter_context(tc.tile_pool(name="psum_r", bufs=1, space="PSUM"))

    # resident weights (convs is DMA'd in 4 per-expert chunks, emitted
    # after sample 0's x DMA so the first image load isn't queued behind it)
    convs_sb = const_pool.tile([CIN, E * TAPCO], F32R)
    w1t_sb = const_pool.tile([CIN, R], F32)
    nc.sync.dma_start(w1t_sb[:], w1t_d[:])
    b1_sb = const_pool.tile([R, 1], F32)
    nc.sync.dma_start(b1_sb[:], b1_d[:])
    w2t_sb = const_pool.tile([R, E], F32)
    nc.sync.dma_start(w2t_sb[:], w2t_d[:])
    g_sb = const_pool.tile([1, E], F32)
    nc.sync.dma_start(g_sb[:], g_d[:])
    ones_sb = const_pool.tile([1, CIN], F32)
    nc.vector.memset(ones_sb[:], 1.0)

    # warmup matmuls: absorb the router-weight DMA waits into PE's clock
    # so the per-sample router matmuls need only their ACT dependency.
    warm1 = psum_r_pool.tile([R, R], F32, tag="warm1")
    nc.tensor.matmul(warm1[:], lhsT=w1t_sb[:, 0:R], rhs=w1t_sb[:, 0:R],
                     start=True, stop=True)
    warm2 = psum_r_pool.tile([E, E], F32, tag="warm2")
    nc.tensor.matmul(warm2[:], lhsT=w2t_sb[:, 0:E], rhs=w2t_sb[:, 0:E],
                     start=True, stop=True)


    # two persistent zero-padded fp32r images, borders zeroed once via DVE
    # copies from an f32 zero row (memset cannot target f32r directly, and
    # the f32->f32r copy is a legal "rounded" producer for the matmuls)
    zrow = const_pool.tile([CIN, WP], F32)
    nc.vector.memset(zrow[:], 0.0)
    xprs = []
    for i in range(2):
        if CONTIG_RHS:
            # flat [1-elem guard][58x58 padded image][1-elem guard]
            tf = const_pool.tile([CIN, HP * WP + 2], F32R, tag=f"xpr{i}")
            t = tf[:, 1 : 1 + HP * WP].rearrange("p (r c) -> p r c", c=WP)
            nc.vector.tensor_copy(tf[:, 0:1], zrow[:, 0:1])
            nc.vector.tensor_copy(tf[:, HP * WP + 1 : HP * WP + 2], zrow[:, 0:1])
        else:
            t = const_pool.tile([CIN, HP, WP], F32R, tag=f"xpr{i}")
            tf = None
        nc.vector.tensor_copy(t[:, 0, :], zrow[:])
        nc.vector.tensor_copy(t[:, HP - 1, :], zrow[:])
        nc.vector.tensor_copy(t[:, 1 : HP - 1, 0], zrow[:, 0 : HP - 2])
        nc.vector.tensor_copy(t[:, 1 : HP - 1, WP - 1], zrow[:, 0 : HP - 2])
        xprs.append((t, tf))

    def prep(b):
        """Per-sample producer work (DMA/ACT/DVE + tiny router matmuls):
        image load, GAP, padded-image copy, router, expert mixing."""
        xp = xp_pool.tile([CIN, H, W], F32)
        nc.sync.dma_start(xp[:], x_d[b])
        if b == 0:
            # expert bank in per-expert chunks, queued after the first image
            for e in range(E):
                nc.sync.dma_start(convs_sb[:, e * TAPCO : (e + 1) * TAPCO],
                                  convs_d[:, e * TAPCO : (e + 1) * TAPCO])

        # GAP on DVE (keeps ACT free for the PSUM stage copies)
        gap = small_pool.tile([CIN, 1], F32, tag="gap")
        nc.vector.reduce_sum(gap[:], xp[:], axis=mybir.AxisListType.XY)

        # materialize padded fp32r image through DVE (f32 -> f32r copy)
        xpr, xpr_flat = xprs[b % 2]
        nc.vector.tensor_copy(xpr[:, 1 : H + 1, 1 : W + 1], xp[:])

        # router (PE deps: ACT only)
        ph = psum_r_pool.tile([R, 1], F32, tag="ph")
        nc.tensor.matmul(ph[:], lhsT=w1t_sb[:], rhs=gap[:], start=True, stop=True)
        hmid = small_pool.tile([R, 1], F32, tag="hmid")
        nc.scalar.activation(hmid[:], ph[:], mybir.ActivationFunctionType.Relu,
                             bias=b1_sb[:], scale=1.0)
        pl = psum_r_pool.tile([1, E], F32, tag="pl")
        nc.tensor.matmul(pl[:], lhsT=hmid[:], rhs=w2t_sb[:],
                         start=True, stop=True)

        # softmax(logits/TEMP + b2/TEMP): logits here are O(0.1) by
        # construction (router weights ~0.05, gap ~N(0, 1/sqrt(HW))), so no
        # max-shift is needed; b2 enters as the constant factor exp(b2/TEMP).
        ex = small_pool.tile([1, E], F32, tag="ex")
        nc.scalar.activation(ex[:], pl[:], mybir.ActivationFunctionType.Exp,
                             scale=1.0 / TEMP)
        exg = small_pool.tile([1, E], F32, tag="exg")
        nc.vector.tensor_mul(exg[:], ex[:], g_sb[:])
        ssum = small_pool.tile([1, 1], F32, tag="ssum")
        nc.vector.reduce_sum(ssum[:], exg[:], axis=mybir.AxisListType.X)
        rec = small_pool.tile([1, 1], F32, tag="rec")
        nc.vector.reciprocal(rec[:], ssum[:])
        rt = small_pool.tile([1, E], F32, tag="rt")
        nc.vector.tensor_scalar_mul(rt[:], exg[:], rec[:])
        # broadcast routing weights to all 128 partitions: ones[1,128].T @ rt
        rb = psum_r_pool.tile([CIN, E], F32, tag="rb")
        nc.tensor.matmul(rb[:], lhsT=ones_sb[:], rhs=rt[:], start=True, stop=True)

        # mix expert kernels: kern[ci, tap, co] = sum_e r[e]*convs[ci,e,tap,co]
        # kern is float32r (rounded on DVE write) so the conv matmuls see a
        # rounded producer; DVE reads use f32 bitcasts of the same bits.
        kern = kern_pool.tile([CIN, TAPCO], F32R)
        nc.vector.tensor_scalar_mul(kern[:], convs_sb[:, 0:TAPCO].bitcast(F32),
                                    rb[:, 0:1])
        for e in range(1, E):
            nc.vector.scalar_tensor_tensor(
                kern[:], convs_sb[:, e * TAPCO : (e + 1) * TAPCO].bitcast(F32),
                rb[:, e : e + 1], kern[:].bitcast(F32),
                op0=mybir.AluOpType.mult, op1=mybir.AluOpType.add)
        return (xpr, xpr_flat), kern

    def conv_half(b, half, xprpair, kern):
        """One Cout-half of the conv: 7 row tiles x 9 taps accumulated in
        PSUM, staged to SBUF on ACT, then DMA'd out."""
        xpr, xpr_flat = xprpair
        stage = stage_pool.tile([128, H, W], F32)
        for t in range(NTILES):
            r0 = ROWS_PER_TILE * t
            if CONTIG_RHS:
                # 8 full padded rows; junk columns discarded at extraction
                ps = psum_pool.tile([128, ROWS_PER_TILE, WP], F32)
                base = 1 + (1 + r0) * WP
                for ki, (dh, dw) in enumerate(TAPS):
                    lhsT = kern[:, ki * COUT + half * 128
                                : ki * COUT + half * 128 + 128]
                    off = base + dh * WP + dw
                    rhs = xpr_flat[:, off : off + NFREE_C]
                    nc.tensor.matmul(ps[:], lhsT=lhsT, rhs=rhs,
                                     start=(ki == 0), stop=(ki == len(TAPS) - 1))
                nc.scalar.copy(stage[:, r0 : r0 + ROWS_PER_TILE, :],
                               ps[:, :, 1 : W + 1])
            else:
                ps = psum_pool.tile([128, ROWS_PER_TILE, W], F32)
                for ki, (dh, dw) in enumerate(TAPS):
                    lhsT = kern[:, ki * COUT + half * 128
                                : ki * COUT + half * 128 + 128]
                    rhs = xpr[:, 1 + r0 + dh : 1 + r0 + dh + ROWS_PER_TILE,
                              1 + dw : 1 + dw + W]
                    nc.tensor.matmul(ps[:], lhsT=lhsT, rhs=rhs,
                                     start=(ki == 0), stop=(ki == len(TAPS) - 1))
                nc.scalar.copy(stage[:, r0 : r0 + ROWS_PER_TILE, :], ps[:])
        nc.sync.dma_start(out_d[b, half * 128 : half * 128 + 128], stage[:])

    # software pipeline: emit sample b+1's producer work between sample b's
    # two conv halves so the mixing for b+1 overlaps b's matmuls instead of
    # serializing behind them (engines execute their streams in order).
    state = prep(0)
    for b in range(BL):
        xprpair, kern = state
        conv_half(b, 0, xprpair, kern)
        if b + 1 < BL:
            state = prep(b + 1)
        conv_half(b, 1, xprpair, kern)


_PROGRAM = None


def round_fp32r(a: np.ndarray) -> np.ndarray:
    """Round fp32 to the fp32r grid: RNE to 11 mantissa bits (top 20 bits
    kept, low 12 zero) — matches walrus's fp32_to_fp32r/fp32r_to_fp32."""
    u = np.ascontiguousarray(a, dtype=np.float32).view(np.uint32)
    drop = 12
    lsb = (u >> drop) & np.uint32(1)
    r = u + (np.uint32((1 << (drop - 1)) - 1) + lsb)
    r &= np.uint32(0xFFFFFFFF) ^ np.uint32((1 << drop) - 1)
    return r.view(np.float32)


def kernel(x, convs, w1, b1, w2, b2):
    global _PROGRAM, LAST_RESULTS
    x = np.ascontiguousarray(np.asarray(x, dtype=np.float32))
    convs = np.asarray(convs, dtype=np.float32)
    w1 = np.asarray(w1, dtype=np.float32)
    b1 = np.asarray(b1, dtype=np.float32)
    w2 = np.asarray(w2, dtype=np.float32)
    b2 = np.asarray(b2, dtype=np.float32)

    if _PROGRAM is None:
        _PROGRAM = _build_program()
    nc = _PROGRAM

    # host-side layout prep (permutes + fp32r grid rounding)
    x = round_fp32r(x)
    convs_r = round_fp32r(
        np.ascontiguousarray(convs.transpose(2, 0, 3, 4, 1)).reshape(CIN, E * TAPCO))
    w1t = np.ascontiguousarray(w1.T) / float(HWN)
    b1c = np.ascontiguousarray(b1[:, None])
    w2t = np.ascontiguousarray(w2.T)
    g = np.ascontiguousarray(np.exp(b2 / TEMP)[None, :]).astype(np.float32)

    in_maps = [
        {
            "x": np.ascontiguousarray(x[c * BL : (c + 1) * BL]),
            "convs": convs_r,
            "w1t": w1t,
            "b1": b1c,
            "w2t": w2t,
            "g": g,
        }
        for c in range(NCORES)
    ]
    res = run_bass_kernel_spmd(nc, in_maps, core_ids=list(range(NCORES)), trace=TRACE)
    LAST_RESULTS = res
    return np.concatenate([res.results[c]["out"] for c in range(NCORES)], axis=0)


# revision 56
# speedup vs baseline: 25.1713x; 25.1713x over previous
"""DyConv (MoE-routed dynamic convolution) Trainium2 Bass kernel.

Data-parallel over batch: 32 samples -> 8 cores x 4 samples.
Per sample, fully on-device:
  gap  = mean(x, HW)                  (VectorE reduce, 1/HW folded into w1)
  h    = relu(gap @ w1.T + b1)        (TensorE matmul K=Cin=128, ScalarE relu)
  l    = h @ w2.T                     (TensorE matmul K=16)
  r    = softmax(l/30 + b2/30)        (ScalarE exp, b2 as const factor
                                       exp(b2/30) on DVE, DVE recip+scale)
  kern = sum_e r[e] * convs[e]        (VectorE scalar_tensor_tensor FMAs)
  out  = conv2d(x, kern, pad=1)       (9 shifted float32r matmuls accumulated
                                       in PSUM; Cin=128 partition contraction,
                                       Cout = 2 halves of 128, 7 row tiles of
                                       N=8*56=448)

The emission is software-pipelined: sample b+1's producer work (image load,
GAP, router, expert mixing) is emitted between sample b's two conv halves so
DVE/ACT prep overlaps PE matmuls. float32r (fp32 rounded to 11 mantissa
bits, TF32-like) runs the PE at 1 cycle/row — 4x faster than fp32 — with
~2e-4 relative output error.
"""

import os
from contextlib import ExitStack

import numpy as np

import concourse.bass as bass
import concourse.bacc as bacc
import concourse.tile as tile
from concourse import mybir
from concourse.bass_utils import run_bass_kernel_spmd

F32 = mybir.dt.float32
F32R = mybir.dt.float32r

B, CIN, H, W = 32, 128, 56, 56
COUT, KS, E, R = 256, 3, 4, 16
NCORES = 8
BL = B // NCORES  # samples per core
TEMP = 30.0
HP, WP = H + 2, W + 2  # zero-padded image dims in SBUF
HWN = H * W  # 3136
ROWS_PER_TILE = 8
NTILES = H // ROWS_PER_TILE  # 7
NFREE = ROWS_PER_TILE * W  # 448 fp32 <= 512 (one PSUM bank)
TAPCO = KS * KS * COUT  # 2304, per-expert slice [tap, co]

# taps in kh-major order, matching the [ci, e, kh, kw, co] host layout
TAPS = [(dh, dw) for dh in (-1, 0, 1) for dw in (-1, 0, 1)]

# module-level knobs for test.py
TRACE = os.environ.get("DYCONV_TRACE", "0") == "1"
LAST_RESULTS = None
MM_DTYPE = F32R
# benchmarking: wrap the whole kernel body in a For_i loop of this many
# iterations (one NEFF, repeated device-side) so wall-clock timing is
# dominated by device time, not axon dispatch RTT.
LOOP_REPS = int(os.environ.get("DYCONV_LOOP_REPS", "1"))
# conv rhs addressing: 0 = strided [8 rows x 56] views of the padded image,
# 1 = fully contiguous 464-element windows over flat padded rows (guard
# elements at both ends keep all 9 tap offsets in-bounds)
CONTIG_RHS = os.environ.get("DYCONV_CONTIG", "0") == "1"
# bisection modes for HW timing probes: full | noout (skip out DMAs) |
# peonly (single prep, no stage copies, no out DMAs - pure matmul stream)
MODE = os.environ.get("DYCONV_MODE", "full")
NFREE_C = ROWS_PER_TILE * WP  # 464 fp32 <= 512
# conv matmul dtype: fp32r (default, ~2e-4 err) or bf16 (~3e-3 err, FWL
# weight loads + DVE fast modes)
BF16 = os.environ.get("DYCONV_BF16", "0") == "1"
# tap-outer blocking: 3-row-tile blocks share each stationary weight across
# 3 consecutive matmuls (tests whether HW/walrus elides redundant weight
# loads). Requires merging router PSUMs to one bank (6 conv banks needed).
TAPOUTER = os.environ.get("DYCONV_TAPOUTER", "0") == "1"


def _build_program():
    # Bacc (not raw Bass): its compile() runs move_matmul_waits_to_ldweights
    # + generate_event_semaphores, legalizing instructions that need more
    # than one hardware sync-wait slot.
    nc = bacc.Bacc("TRN2", target_bir_lowering=False, debug=False)
    # x and convs feed float32r matmuls; host pre-rounds both to the fp32r
    # grid (RNE to 11 mantissa bits) so every on-chip conversion to f32r is
    # value-preserving. The on-chip f32r producers (DVE copy / mixing) are
    # what satisfies the BIR verifier's rounded-producer rule.
    MMDT = mybir.dt.bfloat16 if BF16 else F32R
    xdt = mybir.dt.bfloat16 if BF16 else F32
    x_d = nc.dram_tensor("x", [BL, CIN, H, W], xdt, kind="ExternalInput").ap()
    # host-prearranged: convs_r[ci, e, kh, kw, co] flattened to [128, E*9*COUT]
    convs_d = nc.dram_tensor("convs", [CIN, E * TAPCO], MMDT, kind="ExternalInput").ap()
    # w1.T / (H*W)  -> [CIN, R]
    w1t_d = nc.dram_tensor("w1t", [CIN, R], F32, kind="ExternalInput").ap()
    b1_d = nc.dram_tensor("b1", [R, 1], F32, kind="ExternalInput").ap()
    # w2.T -> [R, E]; g = exp(b2/TEMP) -> [1, E]: the bias enters softmax
    # as a constant per-expert multiplicative factor applied after exp.
    w2t_d = nc.dram_tensor("w2t", [R, E], F32, kind="ExternalInput").ap()
    g_d = nc.dram_tensor("g", [1, E], F32, kind="ExternalInput").ap()
    out_d = nc.dram_tensor("out", [BL, COUT, H, W], F32, kind="ExternalOutput").ap()

    global _MMDT, _XDT
    _MMDT, _XDT = MMDT, xdt
    with tile.TileContext(nc) as tc, ExitStack() as ctx:
        if LOOP_REPS > 1:
            with tc.For_i(0, LOOP_REPS, 1, hint_engines=(mybir.EngineType.PE,)):
                _emit(ctx, tc, x_d, convs_d, w1t_d, b1_d, w2t_d, g_d, out_d)
        else:
            _emit(ctx, tc, x_d, convs_d, w1t_d, b1_d, w2t_d, g_d, out_d)
    nc.compile()
    return nc


def _emit(ctx, tc, x_d, convs_d, w1t_d, b1_d, w2t_d, g_d, out_d):
    # The fp32/fp32r matmul hardware encoding (fused 4-byte weight load,
    # S3_LW) carries at most ONE sync wait, so every matmul is arranged to
    # depend on a single engine's semaphore:
    #   conv matmuls  -> DVE only (kern mixing, padded-image copy, PSUM
    #                    bank release via DVE stage copies)
    #   router mm1/mm2-> ACT only (gap via ACT accumulate, relu on ACT)
    #   rb broadcast  -> DVE only
    # One-time DMA waits for the router weights are absorbed by warmup
    # matmuls that read only those tiles.
    nc = tc.nc

    const_pool = ctx.enter_context(tc.tile_pool(name="const", bufs=1))
    xp_pool = ctx.enter_context(tc.tile_pool(name="xpad", bufs=3))
    kern_pool = ctx.enter_context(tc.tile_pool(name="kern", bufs=2))
    small_pool = ctx.enter_context(tc.tile_pool(name="small", bufs=2))
    stage_pool = ctx.enter_context(tc.tile_pool(name="stage", bufs=4))
    psum_pool = ctx.enter_context(tc.tile_pool(
        name="psum", bufs=6 if TAPOUTER else 3, space="PSUM"))
    psum_r_pool = ctx.enter_context(tc.tile_pool(name="psum_r", bufs=1, space="PSUM"))

    # resident weights (convs is DMA'd in 4 per-expert chunks, emitted
    # after sample 0's x DMA so the first image load isn't queued behind it)
    convs_sb = const_pool.tile([CIN, E * TAPCO], _MMDT)
    w1t_sb = const_pool.tile([CIN, R], F32)
    nc.sync.dma_start(w1t_sb[:], w1t_d[:])
    b1_sb = const_pool.tile([R, 1], F32)
    nc.sync.dma_start(b1_sb[:], b1_d[:])
    w2t_sb = const_pool.tile([R, E], F32)
    nc.sync.dma_start(w2t_sb[:], w2t_d[:])
    g_sb = const_pool.tile([1, E], F32)
    nc.sync.dma_start(g_sb[:], g_d[:])
    ones_sb = const_pool.tile([1, CIN], F32)
    nc.vector.memset(ones_sb[:], 1.0)

    # warmup matmuls: absorb the router-weight DMA waits into PE's clock
    # so the per-sample router matmuls need only their ACT dependency.
    rtag = (lambda s: "rpsum") if TAPOUTER else (lambda s: s)
    warm1 = psum_r_pool.tile([R, R], F32, tag=rtag("warm1"))
    nc.tensor.matmul(warm1[:], lhsT=w1t_sb[:, 0:R], rhs=w1t_sb[:, 0:R],
                     start=True, stop=True)
    warm2 = psum_r_pool.tile([E, E], F32, tag=rtag("warm2"))
    nc.tensor.matmul(warm2[:], lhsT=w2t_sb[:, 0:E], rhs=w2t_sb[:, 0:E],
                     start=True, stop=True)


    # two persistent zero-padded fp32r images, borders zeroed once via DVE
    # copies from an f32 zero row (memset cannot target f32r directly, and
    # the f32->f32r copy is a legal "rounded" producer for the matmuls)
    zrow = const_pool.tile([CIN, WP], F32)
    nc.vector.memset(zrow[:], 0.0)
    xprs = []
    for i in range(2):
        if CONTIG_RHS:
            # flat [1-elem guard][58x58 padded image][1-elem guard]
            tf = const_pool.tile([CIN, HP * WP + 2], _MMDT, tag=f"xpr{i}")
            t = tf[:, 1 : 1 + HP * WP].rearrange("p (r c) -> p r c", c=WP)
            nc.vector.tensor_copy(tf[:, 0:1], zrow[:, 0:1])
            nc.vector.tensor_copy(tf[:, HP * WP + 1 : HP * WP + 2], zrow[:, 0:1])
        else:
            t = const_pool.tile([CIN, HP, WP], _MMDT, tag=f"xpr{i}")
            tf = None
        nc.vector.tensor_copy(t[:, 0, :], zrow[:])
        nc.vector.tensor_copy(t[:, HP - 1, :], zrow[:])
        nc.vector.tensor_copy(t[:, 1 : HP - 1, 0], zrow[:, 0 : HP - 2])
        nc.vector.tensor_copy(t[:, 1 : HP - 1, WP - 1], zrow[:, 0 : HP - 2])
        xprs.append((t, tf))

    def prep(b):
        """Per-sample producer work (DMA/ACT/DVE + tiny router matmuls):
        image load, GAP, padded-image copy, router, expert mixing."""
        xp = xp_pool.tile([CIN, H, W], _XDT)
        nc.sync.dma_start(xp[:], x_d[b])
        if b == 0:
            # expert bank in per-expert chunks, queued after the first image
            for e in range(E):
                nc.sync.dma_start(convs_sb[:, e * TAPCO : (e + 1) * TAPCO],
                                  convs_d[:, e * TAPCO : (e + 1) * TAPCO])

        # GAP on DVE (keeps ACT free for the PSUM stage copies)
        gap = small_pool.tile([CIN, 1], F32, tag="gap")
        nc.vector.reduce_sum(gap[:], xp[:], axis=mybir.AxisListType.XY)

        # materialize padded fp32r image through DVE (f32 -> f32r copy)
        xpr, xpr_flat = xprs[b % 2]
        nc.vector.tensor_copy(xpr[:, 1 : H + 1, 1 : W + 1], xp[:])

        # router (PE deps: ACT only)
        ph = psum_r_pool.tile([R, 1], F32, tag=rtag("ph"))
        nc.tensor.matmul(ph[:], lhsT=w1t_sb[:], rhs=gap[:], start=True, stop=True)
        hmid = small_pool.tile([R, 1], F32, tag="hmid")
        nc.scalar.activation(hmid[:], ph[:], mybir.ActivationFunctionType.Relu,
                             bias=b1_sb[:], scale=1.0)
        pl = psum_r_pool.tile([1, E], F32, tag=rtag("pl"))
        nc.tensor.matmul(pl[:], lhsT=hmid[:], rhs=w2t_sb[:],
                         start=True, stop=True)

        # softmax(logits/TEMP + b2/TEMP): logits here are O(0.1) by
        # construction (router weights ~0.05, gap ~N(0, 1/sqrt(HW))), so no
        # max-shift is needed; b2 enters as the constant factor exp(b2/TEMP).
        ex = small_pool.tile([1, E], F32, tag="ex")
        nc.scalar.activation(ex[:], pl[:], mybir.ActivationFunctionType.Exp,
                             scale=1.0 / TEMP)
        exg = small_pool.tile([1, E], F32, tag="exg")
        nc.vector.tensor_mul(exg[:], ex[:], g_sb[:])
        ssum = small_pool.tile([1, 1], F32, tag="ssum")
        nc.vector.reduce_sum(ssum[:], exg[:], axis=mybir.AxisListType.X)
        rec = small_pool.tile([1, 1], F32, tag="rec")
        nc.vector.reciprocal(rec[:], ssum[:])
        rt = small_pool.tile([1, E], F32, tag="rt")
        nc.vector.tensor_scalar_mul(rt[:], exg[:], rec[:])
        # broadcast routing weights to all 128 partitions: ones[1,128].T @ rt
        rb = psum_r_pool.tile([CIN, E], F32, tag=rtag("rb"))
        nc.tensor.matmul(rb[:], lhsT=ones_sb[:], rhs=rt[:], start=True, stop=True)

        # mix expert kernels: kern[ci, tap, co] = sum_e r[e]*convs[ci,e,tap,co]
        # kern is float32r (rounded on DVE write) so the conv matmuls see a
        # rounded producer; DVE reads use f32 bitcasts of the same bits.
        kern = kern_pool.tile([CIN, TAPCO], _MMDT)
        def rd(ap):
            return ap if BF16 else ap.bitcast(F32)
        nc.vector.tensor_scalar_mul(kern[:], rd(convs_sb[:, 0:TAPCO]),
                                    rb[:, 0:1])
        for e in range(1, E):
            nc.vector.scalar_tensor_tensor(
                kern[:], rd(convs_sb[:, e * TAPCO : (e + 1) * TAPCO]),
                rb[:, e : e + 1], rd(kern[:]),
                op0=mybir.AluOpType.mult, op1=mybir.AluOpType.add)
        return (xpr, xpr_flat), kern

    def conv_half(b, half, xprpair, kern):
        """One Cout-half of the conv: 7 row tiles x 9 taps accumulated in
        PSUM, staged to SBUF on ACT, then DMA'd out."""
        xpr, xpr_flat = xprpair
        if MODE != "peonly":
            stage = stage_pool.tile([128, H, W], F32, tag="stage")
        else:
            stage = None
        for t in range(NTILES):
            r0 = ROWS_PER_TILE * t
            if CONTIG_RHS:
                # 8 full padded rows; junk columns discarded at extraction
                ps = psum_pool.tile([128, ROWS_PER_TILE, WP], F32)
                base = 1 + (1 + r0) * WP
                for ki, (dh, dw) in enumerate(TAPS):
                    lhsT = kern[:, ki * COUT + half * 128
                                : ki * COUT + half * 128 + 128]
                    off = base + dh * WP + dw
                    rhs = xpr_flat[:, off : off + NFREE_C]
                    nc.tensor.matmul(ps[:], lhsT=lhsT, rhs=rhs,
                                     start=(ki == 0), stop=(ki == len(TAPS) - 1))
                nc.scalar.copy(stage[:, r0 : r0 + ROWS_PER_TILE, :],
                               ps[:, :, 1 : W + 1])
            else:
                ps = psum_pool.tile([128, ROWS_PER_TILE, W], F32)
                for ki, (dh, dw) in enumerate(TAPS):
                    lhsT = kern[:, ki * COUT + half * 128
                                : ki * COUT + half * 128 + 128]
                    rhs = xpr[:, 1 + r0 + dh : 1 + r0 + dh + ROWS_PER_TILE,
                              1 + dw : 1 + dw + W]
                    nc.tensor.matmul(ps[:], lhsT=lhsT, rhs=rhs,
                                     start=(ki == 0), stop=(ki == len(TAPS) - 1))
                if MODE != "peonly":
                    nc.scalar.copy(stage[:, r0 : r0 + ROWS_PER_TILE, :], ps[:])
        if MODE == "full":
            nc.sync.dma_start(out_d[b, half * 128 : half * 128 + 128], stage[:])

    def conv_half_tapouter(b, half, xprpair, kern):
        """Tap-outer variant: blocks of 3 row tiles accumulate in 3 PSUM
        banks while each tap's stationary weight is reused across the block
        (same per-element accumulation order as conv_half)."""
        xpr, _ = xprpair
        if MODE != "peonly":
            stage = stage_pool.tile([128, H, W], F32, tag="stage")
        for t0 in range(0, NTILES, 3):
            ts = list(range(t0, min(t0 + 3, NTILES)))
            pss = []
            for t in ts:
                ps = psum_pool.tile([128, ROWS_PER_TILE, W], F32, tag="psum")
                pss.append(ps)
            for ki, (dh, dw) in enumerate(TAPS):
                lhsT = kern[:, ki * COUT + half * 128
                            : ki * COUT + half * 128 + 128]
                for j, t in enumerate(ts):
                    r0 = ROWS_PER_TILE * t
                    rhs = xpr[:, 1 + r0 + dh : 1 + r0 + dh + ROWS_PER_TILE,
                              1 + dw : 1 + dw + W]
                    nc.tensor.matmul(pss[j][:], lhsT=lhsT, rhs=rhs,
                                     start=(ki == 0), stop=(ki == len(TAPS) - 1))
            if MODE != "peonly":
                for j, t in enumerate(ts):
                    r0 = ROWS_PER_TILE * t
                    nc.scalar.copy(stage[:, r0 : r0 + ROWS_PER_TILE, :], pss[j][:])
        if MODE == "full":
            nc.sync.dma_start(out_d[b, half * 128 : half * 128 + 128], stage[:])

    # software pipeline: emit sample b+1's producer work between sample b's
    # two conv halves so the mixing for b+1 overlaps b's matmuls instead of
    # serializing behind them (engines execute their streams in order).
    ch = conv_half_tapouter if TAPOUTER else conv_half
    state = prep(0)
    for b in range(BL):
        xprpair, kern = state
        ch(b, 0, xprpair, kern)
        if b + 1 < BL and MODE != "peonly":
            state = prep(b + 1)
        ch(b, 1, xprpair, kern)
    if MODE != "full":
        # probe modes produce no real output; write something so the
        # ExternalOutput tensor has a producer
        dummy_stage = stage_pool.tile([128, H, W], F32, tag="stage")
        nc.scalar.memzero(dummy_stage[:])
        nc.sync.dma_start(out_d[0, 0:128], dummy_stage[:])


_PROGRAM = None
_MMDT = F32R
_XDT = F32


def round_fp32r(a: np.ndarray) -> np.ndarray:
    """Round fp32 to the fp32r grid: RNE to 11 mantissa bits (top 20 bits
    kept, low 12 zero) — matches walrus's fp32_to_fp32r/fp32r_to_fp32."""
    u = np.ascontiguousarray(a, dtype=np.float32).view(np.uint32)
    drop = 12
    lsb = (u >> drop) & np.uint32(1)
    r = u + (np.uint32((1 << (drop - 1)) - 1) + lsb)
    r &= np.uint32(0xFFFFFFFF) ^ np.uint32((1 << drop) - 1)
    return r.view(np.float32)


def kernel(x, convs, w1, b1, w2, b2):
    global _PROGRAM, LAST_RESULTS
    x = np.ascontiguousarray(np.asarray(x, dtype=np.float32))
    convs = np.asarray(convs, dtype=np.float32)
    w1 = np.asarray(w1, dtype=np.float32)
    b1 = np.asarray(b1, dtype=np.float32)
    w2 = np.asarray(w2, dtype=np.float32)
    b2 = np.asarray(b2, dtype=np.float32)

    if _PROGRAM is None:
        _PROGRAM = _build_program()
    nc = _PROGRAM

    # host-side layout prep (permutes + device-dtype rounding)
    if BF16:
        import ml_dtypes
        x = x.astype(ml_dtypes.bfloat16)
        convs_r = np.ascontiguousarray(
            convs.transpose(2, 0, 3, 4, 1)).reshape(CIN, E * TAPCO).astype(
            ml_dtypes.bfloat16)
    else:
        x = round_fp32r(x)
        convs_r = round_fp32r(
            np.ascontiguousarray(convs.transpose(2, 0, 3, 4, 1)).reshape(CIN, E * TAPCO))
    w1t = np.ascontiguousarray(w1.T) / float(HWN)
    b1c = np.ascontiguousarray(b1[:, None])
    w2t = np.ascontiguousarray(w2.T)
    g = np.ascontiguousarray(np.exp(b2 / TEMP)[None, :]).astype(np.float32)

    in_maps = [
        {
            "x": np.ascontiguousarray(x[c * BL : (c + 1) * BL]),
            "convs": convs_r,
            "w1t": w1t,
            "b1": b1c,
            "w2t": w2t,
            "g": g,
        }
        for c in range(NCORES)
    ]
    res = run_bass_kernel_spmd(nc, in_maps, core_ids=list(range(NCORES)), trace=TRACE)
    LAST_RESULTS = res
    return np.concatenate([res.results[c]["out"] for c in range(NCORES)], axis=0)
